# revision 12
# baseline (speedup 1.0000x reference)
"""Sparse (graph-masked) multi-head attention on 8 Trainium2 NeuronCores.

Reference computation (fp32, single device):
    qkv = x @ w_qkv + b_qkv ; split heads (H=8, D=64)
    scores = q k^T / sqrt(D), masked by adj_matrix (True=attend)
    y = softmax(scores) @ v ; out = y @ w_proj + b_proj

Sharding: core = (batch b, query-half th).  Each core owns queries
t in [th*1024, (th+1)*1024) of batch b and produces out[b, that slice, :].
No cross-core communication.

v2 design (fused single-pass, ACT-saturating):
  The kernel is exp-bound: 8 heads x 1024 q x 2048 k = 16.8M exps/core on
  the Scalar engine at ~1 elem/lane/cycle = ~128us busy.  Everything else
  (projections, score/y matmuls, mask multiply, normalize, out-proj) is
  scheduled to hide under that:
  - DMA loads are priority-ordered and spread over the sync (HWDGE) and
    gpsimd (SWDGE) queues; the mask streams in 4 groups during attention.
  - A dummy exp at t=0 preloads the ACT spline table during the DMA wait.
  - Attention runs pair-at-a-time (single chain): psA (scores) bufs=2
    pipelines exp back-to-back; PSUM plan ps1(2)+psA(4)+psY(2) = 8 banks
    lets projections, attention and out-proj coexist.
  - Later pairs' q/k projections and tb0's output projection are emitted
    as fillers inside earlier attention chunk loops (PE gap-fill).
  - Mask multiply (exp out x adjacency) alternates DVE / Pool engines.
  - Softmax denominators come free as a ones-column in v (65th col per
    head); y tiles leave PSUM via DMA, denominators get a batched
    reciprocal_approx_fast, Pool broadcasts, DVE normalizes straight into
    the packed head-pair tiles the output projection consumes.
"""

import numpy as np
import ml_dtypes

import concourse.bass as bass
import concourse.mybir as mybir
import concourse.tile as tile
from concourse import bacc
from concourse.bass_utils import run_bass_kernel_spmd

BF16 = mybir.dt.bfloat16
F32 = mybir.dt.float32
nbf16 = ml_dtypes.bfloat16

B, T, C, H = 4, 2048, 512, 8
D = C // H          # 64
P = 128
NCORES = 8
TL = T // 2         # queries per core
SCALE = 1.0 / float(np.sqrt(D))

AF = mybir.ActivationFunctionType
ALU = mybir.AluOpType


def build_program(t_full=T, t_local=TL, loop_reps=1, num_devices=NCORES,
                  dbg=False):
    """Build the SPMD Bass program (identical on all cores)."""
    nkc = C // P                # 4  contraction chunks over C
    nsc = t_full // P           # 16 key/s chunks
    ntc = t_local // P          # 8  output t chunks
    VW = D + 1                  # v columns per head incl. ones column
    NB = 512                    # PSUM bank free size (f32)
    TB = 512                    # t-block per attention pass
    ntb = t_local // TB         # 2

    nc = bacc.Bacc("TRN2", target_bir_lowering=False, debug=False,
                   num_devices=num_devices)
    dbg_t = {}
    if dbg:
        for name, shape, dt in (
                ("dbg_qT", [P, nkc * t_local], BF16),
                ("dbg_kT", [P, nkc * t_full], BF16),
                ("dbg_v", [P, nsc * H * VW], BF16),
                ("dbg_at0", [P, 2 * TB], BF16),
                ("dbg_am0", [P, 2 * TB], BF16),
                ("dbg_y00", [VW, TB], F32),
                ("dbg_rcp0", [1, TB], F32),
                ("dbg_yp0", [P, t_local], BF16)):
            dbg_t[name] = nc.dram_tensor(name, shape, dt,
                                         kind="ExternalOutput").ap()

    xT = nc.dram_tensor("xT", [C, t_full], BF16, kind="ExternalInput").ap()
    xTq = nc.dram_tensor("xTq", [C, t_local], BF16, kind="ExternalInput").ap()
    maskT = nc.dram_tensor("maskT", [t_full, t_local], BF16,
                           kind="ExternalInput").ap()
    wq = nc.dram_tensor("wq", [C, C], BF16, kind="ExternalInput").ap()
    wk = nc.dram_tensor("wk", [C, C], BF16, kind="ExternalInput").ap()
    wv = nc.dram_tensor("wv", [C, C], BF16, kind="ExternalInput").ap()
    wp = nc.dram_tensor("wp", [C, C], BF16, kind="ExternalInput").ap()
    bqk = nc.dram_tensor("bqk", [P, 2 * nkc], F32, kind="ExternalInput").ap()
    bvp = nc.dram_tensor("bvp", [2, C], F32, kind="ExternalInput").ap()
    out = nc.dram_tensor("out", [t_local, C], F32, kind="ExternalOutput").ap()

    with tile.TileContext(nc) as tc:
        def body():
            with (tc.tile_pool(name="persist", bufs=1) as pp,
                  tc.tile_pool(name="ps1", bufs=2, space="PSUM") as ps1,
                  tc.tile_pool(name="psA", bufs=2, space="PSUM") as psA,
                  tc.tile_pool(name="psY", bufs=1, space="PSUM") as psY,
                  tc.tile_pool(name="atp", bufs=2) as atp,
                  tc.tile_pool(name="amp", bufs=4) as amp,
                  tc.tile_pool(name="ysb", bufs=2) as ysb,
                  tc.tile_pool(name="sm", bufs=2) as sm,
                  tc.tile_pool(name="osb", bufs=2) as osb):
                # ---- ACT table preload (runs during the DMA wait) ----
                zt = sm.tile([1, 8], F32, tag="zt")
                nc.vector.memset(zt[:], 0.0)
                zo = sm.tile([1, 8], BF16, tag="zo")
                nc.scalar.activation(zo[:], zt[:], AF.Exp, scale=1.0)

                # ---- persistent tiles ----
                xT_sb = pp.tile([P, nkc, t_full], BF16, tag="xT")
                xTq_sb = pp.tile([P, nkc, t_local], BF16, tag="xTq")
                mask_sb = pp.tile([P, nsc, t_local], BF16, tag="mask")
                w_sb = {}
                for name in ("wq", "wk", "wv", "wp"):
                    w_sb[name] = pp.tile([P, nkc, C], BF16, tag=name,
                                         name=name)
                bqk_sb = pp.tile([P, 2 * nkc], F32, tag="bqk")
                bv_row = pp.tile([1, C], F32, tag="bv_row")
                bp_row = pp.tile([1, C], F32, tag="bp_row")
                bv_bc = pp.tile([P, C], F32, tag="bv_bc")
                bp_bc = pp.tile([P, C], F32, tag="bp_bc")
                qT_sb = pp.tile([P, nkc, t_local], BF16, tag="qT")
                kT_sb = pp.tile([P, nkc, t_full], BF16, tag="kT")
                v_sb = pp.tile([P, nsc, H * VW], BF16, tag="v")
                yT_pair = [pp.tile([P, t_local], BF16, tag=f"yTp{j}",
                                   name=f"yTp{j}")
                           for j in range(H // 2)]

                # ---- loads: priority order, queue-spread ----
                # sync (HWDGE): weights/x in dependency order
                nc.sync.dma_start(
                    w_sb["wq"][:], wq.rearrange("(k p) c -> p k c", p=P))
                nc.sync.dma_start(
                    xTq_sb[:], xTq.rearrange("(k p) t -> p k t", p=P))
                nc.sync.dma_start(
                    xT_sb[:], xT.rearrange("(k p) t -> p k t", p=P))
                nc.sync.dma_start(
                    w_sb["wk"][:], wk.rearrange("(k p) c -> p k c", p=P))
                nc.sync.dma_start(
                    w_sb["wv"][:], wv.rearrange("(k p) c -> p k c", p=P))
                nc.sync.dma_start(bqk_sb[:], bqk[:])
                nc.sync.dma_start(bv_row[:], bvp[0:1, :])
                nc.sync.dma_start(bp_row[:], bvp[1:2, :])
                nc.sync.dma_start(
                    w_sb["wp"][:], wp.rearrange("(k p) c -> p k c", p=P))
                # gpsimd (SWDGE): mask in 4 groups, consumed progressively
                mask_r = maskT.rearrange("(i p) t -> p i t", p=P)
                ngrp = 4
                gs = nsc // ngrp
                for g in range(ngrp):
                    nc.gpsimd.dma_start(mask_sb[:, g * gs:(g + 1) * gs],
                                        mask_r[:, g * gs:(g + 1) * gs])

                # ones for the per-head denominator columns (overwritten
                # with v on the D data columns by each chunk's STT)
                nc.vector.memset(v_sb[:], 1.0)
                nc.gpsimd.partition_broadcast(bv_bc[:], bv_row[:])
                nc.gpsimd.partition_broadcast(bp_bc[:], bp_row[:])

                # ---- phase-1 emitters (each group: 4 matmuls + cast) ----
                def q_slice(j, sl):
                    tsl = slice(sl * NB, (sl + 1) * NB)
                    pq = ps1.tile([P, NB], F32, tag="p1", name=f"pq{j}_{sl}")
                    for k in range(nkc):
                        nc.tensor.matmul(
                            pq[:], w_sb["wq"][:, k, j * P:(j + 1) * P],
                            xTq_sb[:, k, tsl],
                            start=(k == 0), stop=(k == nkc - 1))
                    nc.vector.tensor_scalar_add(
                        qT_sb[:, j, tsl], pq[:], bqk_sb[:, j:j + 1])

                def k_slice(j, sl):
                    tsl = slice(sl * NB, (sl + 1) * NB)
                    pk = ps1.tile([P, NB], F32, tag="p1", name=f"pk{j}_{sl}")
                    for k in range(nkc):
                        nc.tensor.matmul(
                            pk[:], w_sb["wk"][:, k, j * P:(j + 1) * P],
                            xT_sb[:, k, tsl],
                            start=(k == 0), stop=(k == nkc - 1))
                    nc.vector.tensor_scalar_add(
                        kT_sb[:, j, tsl], pk[:], bqk_sb[:, nkc + j:nkc + j + 1])

                def v_chunk(i):
                    pv = ps1.tile([P, C], F32, tag="p1", name=f"pv{i}")
                    for k in range(nkc):
                        nc.tensor.matmul(
                            pv[:], xT_sb[:, k, i * P:(i + 1) * P],
                            w_sb["wv"][:, k], start=(k == 0),
                            stop=(k == nkc - 1))
                    v_dst = v_sb[:, i].rearrange(
                        "p (h w) -> p h w", w=VW)[:, :, 0:D]
                    nc.vector.scalar_tensor_tensor(
                        v_dst, pv[:].rearrange("p (h d) -> p h d", d=D),
                        0.0, bv_bc[:].rearrange("p (h d) -> p h d", d=D),
                        op0=ALU.add, op1=ALU.add)

                def proj_pair(j):
                    return ([lambda sl=sl: q_slice(j, sl)
                             for sl in range(t_local // NB)] +
                            [lambda sl=sl: k_slice(j, sl)
                             for sl in range(t_full // NB)])

                def out_chunk(tch):
                    po = ps1.tile([P, C], F32, tag="p1", name=f"po{tch}")
                    for j in range(H // 2):
                        nc.tensor.matmul(
                            po[:], yT_pair[j][:, tch * P:(tch + 1) * P],
                            w_sb["wp"][:, j],
                            start=(j == 0), stop=(j == H // 2 - 1))
                    o_sb = osb.tile([P, C], F32, tag="o")
                    nc.vector.scalar_tensor_tensor(
                        o_sb[:], po[:], 0.0, bp_bc[:],
                        op0=ALU.add, op1=ALU.add)
                    nc.sync.dma_start(out[tch * P:(tch + 1) * P, :], o_sb[:])

                # ---- attention: one head-pair x one t-block, 16 s-chunks;
                # `fillers` are PE gap-fill thunks run one per chunk ----
                def attn(p, tb, fillers=()):
                    h0, h1 = 2 * p, 2 * p + 1
                    tsl = slice(tb * TB, (tb + 1) * TB)
                    py0 = psY.tile([VW, TB], F32, tag="y0", name=f"py0_{p}{tb}")
                    py1 = psY.tile([VW, TB], F32, tag="y1", name=f"py1_{p}{tb}")
                    ps_tiles = {}

                    def emit_scores(i):
                        ps = psA.tile([P, 2 * TB], F32, tag="s", name="ps")
                        ps_tiles[i] = ps
                        nc.tensor.matmul(
                            ps[:, 0:TB],
                            kT_sb[0:D, p, i * P:(i + 1) * P],
                            qT_sb[0:D, p, tsl],
                            start=True, stop=True, tile_position=(0, 0))
                        nc.tensor.matmul(
                            ps[:, TB:2 * TB],
                            kT_sb[D:P, p, i * P:(i + 1) * P],
                            qT_sb[D:P, p, tsl],
                            start=True, stop=True, tile_position=(D, 0))

                    emit_scores(0)
                    emit_scores(1)
                    fi = iter(fillers)
                    for i in range(nsc):
                        ps = ps_tiles.pop(i)
                        at = atp.tile([P, 2 * TB], BF16, tag="at")
                        nc.scalar.activation(at[:], ps[:], AF.Exp,
                                             scale=SCALE)
                        am = amp.tile([P, 2 * TB], BF16, tag="am", name="am")
                        mask_bc = mask_sb[:, i, tsl].rearrange(
                            "p (o n) -> p o n", o=1).broadcast_to([P, 2, TB])
                        eng = nc.gpsimd if (i % 4 == 3) else nc.vector
                        eng.tensor_tensor(
                            am[:].rearrange("p (g n) -> p g n", g=2),
                            at[:].rearrange("p (g n) -> p g n", g=2),
                            mask_bc, op=ALU.mult)
                        if dbg and p == 0 and tb == 0 and i == 0:
                            nc.sync.dma_start(dbg_t["dbg_at0"], at[:])
                            nc.sync.dma_start(dbg_t["dbg_am0"], am[:])
                        nc.tensor.matmul(
                            py0[:], v_sb[:, i].rearrange(
                                "p (g w) -> p g w", w=VW)[:, h0],
                            am[:, 0:TB], start=(i == 0), stop=(i == nsc - 1))
                        nc.tensor.matmul(
                            py1[:], v_sb[:, i].rearrange(
                                "p (g w) -> p g w", w=VW)[:, h1],
                            am[:, TB:2 * TB], start=(i == 0),
                            stop=(i == nsc - 1))
                        for f in [next(fi, None)]:
                            if f is not None:
                                f()
                        if i + 2 < nsc:
                            emit_scores(i + 2)

                    # drain y accumulators out of PSUM (frees psY slots),
                    # then normalize into the packed head-pair tile
                    ySB0 = ysb.tile([VW, TB], F32, tag="ySB0")
                    ySB1 = ysb.tile([VW, TB], F32, tag="ySB1")
                    nc.vector.tensor_copy(ySB0[:], py0[:])
                    nc.vector.tensor_copy(ySB1[:], py1[:])
                    # reciprocal_approx_fast requires partition-0 input:
                    # stage the denominator rows down first (SBUF->SBUF)
                    den0 = sm.tile([1, TB], F32, tag="den0")
                    den1 = sm.tile([1, TB], F32, tag="den1")
                    nc.vector.tensor_copy(den0[:], ySB0[D:VW, :])
                    nc.vector.tensor_copy(den1[:], ySB1[D:VW, :])
                    rcp0 = sm.tile([1, TB], F32, tag="rcp0")
                    rcp1 = sm.tile([1, TB], F32, tag="rcp1")
                    nc.vector.reciprocal_approx_fast(rcp0[:], den0[:])
                    nc.vector.reciprocal_approx_fast(rcp1[:], den1[:])
                    rbc0 = sm.tile([D, TB], F32, tag="rbc0")
                    rbc1 = sm.tile([D, TB], F32, tag="rbc1")
                    nc.gpsimd.partition_broadcast(rbc0[:], rcp0[:])
                    nc.gpsimd.partition_broadcast(rbc1[:], rcp1[:])
                    nc.vector.tensor_tensor(
                        yT_pair[p][0:D, tsl], ySB0[0:D, :], rbc0[:],
                        op=ALU.mult)
                    nc.vector.tensor_tensor(
                        yT_pair[p][D:P, tsl], ySB1[0:D, :], rbc1[:],
                        op=ALU.mult)
                    if dbg and p == 0 and tb == 0:
                        nc.sync.dma_start(dbg_t["dbg_y00"], ySB0[:])
                        nc.sync.dma_start(dbg_t["dbg_rcp0"], rcp0[:])

                # ---- emission schedule ----
                q_slice(0, 0)
                q_slice(0, 1)
                for sl in range(t_full // NB):
                    k_slice(0, sl)
                v_chunk(0)
                v_chunk(1)
                # v chunks 2..15 feed pair-0/tb-0's y matmuls just in time
                attn(0, 0, fillers=[lambda i=i: v_chunk(i)
                                    for i in range(2, nsc)])
                attn(0, 1, fillers=proj_pair(1))
                attn(1, 0, fillers=proj_pair(2))
                attn(1, 1, fillers=proj_pair(3))
                attn(2, 0)
                attn(2, 1)
                attn(3, 0)
                # tb-0 output projection hides inside pair-3/tb-1 attention
                attn(3, 1, fillers=[lambda t=t: out_chunk(t)
                                    for t in range(ntc // 2)])
                for tch in range(ntc // 2, ntc):
                    out_chunk(tch)
                if dbg:
                    nc.sync.dma_start(dbg_t["dbg_qT"], qT_sb[:])
                    nc.sync.dma_start(dbg_t["dbg_kT"], kT_sb[:])
                    nc.sync.dma_start(dbg_t["dbg_v"], v_sb[:])
                    nc.sync.dma_start(dbg_t["dbg_yp0"], yT_pair[0][:])

        if loop_reps > 1:
            ET = mybir.EngineType
            with tc.For_i(0, loop_reps, 1,
                          hint_engines=(ET.PE, ET.DVE, ET.Activation,
                                        ET.Pool, ET.SP)):
                body()
        else:
            body()

    nc.compile()
    return nc


def shard_inputs(x, adj_matrix, w_qkv, b_qkv, w_proj, b_proj,
                 t_full=T, t_local=TL):
    """Host-side shard/layout prep. Core c handles (b, th) = divmod(c, 2)."""
    nkc = C // P
    wq = np.ascontiguousarray(w_qkv[:, 0:C]).astype(nbf16)
    wk = np.ascontiguousarray(w_qkv[:, C:2 * C]).astype(nbf16)
    wv = np.ascontiguousarray(w_qkv[:, 2 * C:3 * C]).astype(nbf16)
    wp = np.ascontiguousarray(w_proj).astype(nbf16)
    bq = np.asarray(b_qkv[0:C], dtype=np.float32)
    bk = np.asarray(b_qkv[C:2 * C], dtype=np.float32)
    bv = np.asarray(b_qkv[2 * C:3 * C], dtype=np.float32)
    bp = np.asarray(b_proj, dtype=np.float32)
    # bqk[:, j] = bq[128j:128j+128]; bqk[:, nkc+j] = bk[...]
    bqk = np.concatenate([bq.reshape(nkc, P).T, bk.reshape(nkc, P).T],
                         axis=1)
    bqk = np.ascontiguousarray(bqk, dtype=np.float32)
    bvp = np.ascontiguousarray(np.stack([bv, bp]), dtype=np.float32)
    in_maps = []
    n_th = t_full // t_local
    for core in range(B * n_th):
        b, th = divmod(core, n_th)
        xTb = np.ascontiguousarray(x[b, :t_full].T).astype(nbf16)
        tsl = slice(th * t_local, (th + 1) * t_local)
        in_maps.append({
            "xT": xTb,
            "xTq": np.ascontiguousarray(xTb[:, tsl]),
            "maskT": np.ascontiguousarray(
                adj_matrix[b, :t_full, :t_full].T[:, tsl]).astype(nbf16),
            "wq": wq, "wk": wk, "wv": wv, "wp": wp,
            "bqk": bqk, "bvp": bvp,
        })
    return in_maps


_PROGRAM_CACHE = {}


def _get_program(key=(T, TL, 1)):
    if key not in _PROGRAM_CACHE:
        _PROGRAM_CACHE[key] = build_program(t_full=key[0], t_local=key[1],
                                            loop_reps=key[2])
    return _PROGRAM_CACHE[key]


def kernel(**inputs):
    x = np.asarray(inputs["x"])
    adj = np.asarray(inputs["adj_matrix"])
    nc = _get_program()
    in_maps = shard_inputs(x, adj, np.asarray(inputs["w_qkv"]),
                           np.asarray(inputs["b_qkv"]),
                           np.asarray(inputs["w_proj"]),
                           np.asarray(inputs["b_proj"]))
    res = run_bass_kernel_spmd(nc, in_maps, list(range(NCORES)))
    out = np.empty((B, T, C), dtype=np.float32)
    for core in range(NCORES):
        b, th = divmod(core, 2)
        out[b, th * TL:(th + 1) * TL, :] = res.results[core]["out"]
    return out
